# revision 2
# baseline (speedup 1.0000x reference)
"""EstPop_Debias on 8 Trainium2 NeuronCores.

Strategy (expert/embedding-style parallelism, all-to-all emulated by host
routing): the A/B tables are sharded by item-id range over the 8 cores.
The host routes each sampled item id to its owning shard and converts it
to (tile, partition, column) coordinates; each core streams its table
shard through SBUF in [128 x 1960] tiles and, per tile:

  - builds the updated-slot mask with a GPSIMD local_scatter of the
    routed per-partition column indices,
  - computes delta = (1-a)*B + a*(t1 - A) densely in one fused custom
    DVE op,
  - computes -ln(delta) on the Scalar (ACT) engine,
  - merges delta / t1 into the B / A tiles with predicated copies,
  - streams the merged tiles and the log-weight tile back to HBM.

The host then reassembles the full tables and routes the per-slot log
weights back to the 4M sampled positions (dedup + inverse mapping).
"""

import numpy as np

import concourse.bacc as bacc
import concourse.mybir as mybir
import concourse.tile as tile
from concourse import bass_utils, library_config

ALPHA = 0.0001
ITEM_NUM = 10000001     # table size, matches reference
N_ITEMS = 4194304       # sampled ids per step, matches reference
NS = 8                  # neuron cores
P = 128                 # SBUF partitions
F = 1960                # table columns per partition per tile
TILES = 5               # tiles per shard: 5*128*1960 = 1254400 >= ceil(table/8)
C_DEFAULT = 800         # per-partition routed-index capacity (max observed 761)

AF = mybir.ActivationFunctionType
ALU = mybir.AluOpType

_NC_CACHE = {}
_DELTA_OP = None


def _register_delta_op():
    """Fused EMA delta as a custom DVE op:
    out = s1*in1 + (s0 - imm2*in0), bound as in0=A, in1=B,
    s0 = alpha*(t+1) per-partition scalar, s1 = 1-alpha, imm2 = alpha."""
    global _DELTA_OP
    if _DELTA_OP is not None:
        return _DELTA_OP
    from concourse.dve_ops import (
        OPS, CUSTOM_DVE_SPECS, DveOp, DveOpSpec,
        _CUSTOM_DVE_ROW_BASE, _SUB_OPCODE_FOR_NAME,
    )
    from concourse.dve_spec import Spec, Src0, Src1, C0, C1, C2, lower

    name = "EMA_DELTA_ANT"
    spec = Spec(
        body=Src1 * C1 + (C0 - Src0 * C2),
        reference=lambda in0, in1, s0, s1, imm2: in1 * s1 + (s0 - in0 * imm2),
    )
    if name in _SUB_OPCODE_FOR_NAME:
        _DELTA_OP = next(op for op in OPS if op.name == name)
        return _DELTA_OP
    row = _CUSTOM_DVE_ROW_BASE + len(OPS)
    assert row < 0x20, "custom DVE row table full"
    shas = {}
    for ver in ("v3", "v4"):
        u = lower(spec, ver=ver)
        shas[ver] = DveOpSpec(name=name, opcode=row, uops=u, rd1_en=True).sha(ver)
    op = DveOp(name, spec, subdim=False, uops_sha=shas)
    OPS.append(op)
    CUSTOM_DVE_SPECS[name] = spec
    _SUB_OPCODE_FOR_NAME[name] = row
    _DELTA_OP = op
    return op


def _build_nc(C):
    """One SPMD program; every core runs the same tile loop on its shard."""
    delta_op = _register_delta_op()
    f32, i16, u16, bf16 = (mybir.dt.float32, mybir.dt.int16,
                           mybir.dt.uint16, mybir.dt.bfloat16)
    nc = bacc.Bacc("TRN2", target_bir_lowering=False, debug=False,
                   num_devices=NS)
    Ain_d = nc.dram_tensor("Ain", [TILES * P, F], f32, kind="ExternalInput").ap()
    Bin_d = nc.dram_tensor("Bin", [TILES * P, F], f32, kind="ExternalInput").ap()
    idx_d = nc.dram_tensor("idx", [TILES * P, C], i16, kind="ExternalInput").ap()
    tb_d = nc.dram_tensor("tb", [P, 1], f32, kind="ExternalInput").ap()
    Aout_d = nc.dram_tensor("Aout", [TILES * P, F], f32, kind="ExternalOutput").ap()
    Bout_d = nc.dram_tensor("Bout", [TILES * P, F], f32, kind="ExternalOutput").ap()
    Llog_d = nc.dram_tensor("Llog", [TILES * P, F], bf16, kind="ExternalOutput").ap()

    with tile.TileContext(nc) as tc:
        with tc.tile_pool(name="const", bufs=1) as cpool, \
             tc.tile_pool(name="work", bufs=4) as pool:
            tb = cpool.tile([P, 1], f32)
            c_ap = cpool.tile([P, 1], f32)
            t1_ap = cpool.tile([P, 1], f32)
            ones_t = cpool.tile([P, C], u16)
            T1t = cpool.tile([P, F], f32)
            nc.sync.dma_start(out=tb[:], in_=tb_d[:])
            nc.vector.tensor_scalar(out=t1_ap[:], in0=tb[:], scalar1=1.0,
                                    scalar2=None, op0=ALU.add)
            nc.vector.tensor_scalar(out=c_ap[:], in0=t1_ap[:], scalar1=ALPHA,
                                    scalar2=None, op0=ALU.mult)
            nc.vector.memset(ones_t[:], 1)
            nc.vector.memset(T1t[:], 0.0)
            nc.scalar.activation(out=T1t[:], in_=T1t[:], func=AF.Identity,
                                 bias=t1_ap[:], scale=1.0)
            nc.gpsimd.load_library(library_config.local_scatter)

            for tau in range(TILES):
                sl = slice(tau * P, (tau + 1) * P)
                At = pool.tile([P, F], f32)
                Bt = pool.tile([P, F], f32)
                Dt = pool.tile([P, F], f32)
                Lt = pool.tile([P, F], f32)
                Lb = pool.tile([P, F], bf16)
                Mt = pool.tile([P, F], u16)
                It = pool.tile([P, C], i16)
                nc.sync.dma_start(out=At[:], in_=Ain_d[sl, :])
                nc.sync.dma_start(out=Bt[:], in_=Bin_d[sl, :])
                nc.sync.dma_start(out=It[:], in_=idx_d[sl, :])
                nc.gpsimd.local_scatter(out_ap=Mt[:], data_ap=ones_t[:],
                                        idxs_ap=It[:], channels=P,
                                        num_elems=F, num_idxs=C)
                nc.vector._custom_dve(delta_op, out=Dt[:], in0=At[:], in1=Bt[:],
                                      s0=c_ap[:], s1=1.0 - ALPHA, imm2=ALPHA)
                nc.scalar.activation(out=Lt[:], in_=Dt[:], func=AF.Ln)
                nc.scalar.activation(out=Lb[:], in_=Lt[:], func=AF.Copy,
                                     scale=-1.0)
                nc.vector.copy_predicated(out=Bt[:], mask=Mt[:], data=Dt[:])
                nc.vector.copy_predicated(out=At[:], mask=Mt[:], data=T1t[:])
                nc.scalar.dma_start(out=Aout_d[sl, :], in_=At[:])
                nc.scalar.dma_start(out=Bout_d[sl, :], in_=Bt[:])
                nc.scalar.dma_start(out=Llog_d[sl, :], in_=Lb[:])
    nc.compile()
    return nc


def _get_nc(C):
    nc = _NC_CACHE.get(C)
    if nc is None:
        nc = _build_nc(C)
        _NC_CACHE[C] = nc
    return nc


def kernel(items, A, B, t):
    items = np.asarray(items)
    A = np.asarray(A, dtype=np.float32)
    B = np.asarray(B, dtype=np.float32)
    t = np.asarray(t, dtype=np.float32)
    n_table = A.shape[0]
    assert n_table == ITEM_NUM and B.shape[0] == n_table

    SPAD = TILES * P * F                      # padded slots per shard
    S_OWN = (n_table + NS - 1) // NS          # slots owned per shard
    assert SPAD >= S_OWN

    # ---- host routing: shard by id range, dedup, to local coordinates ----
    uniq, inverse = np.unique(items.astype(np.int64), return_inverse=True)
    assert uniq.min() >= 0 and uniq.max() < n_table
    shard_of = uniq // S_OWN
    lu = uniq - shard_of * S_OWN
    tile_of = lu // (P * F)
    part_of = (lu % (P * F)) // F
    col_of = (lu % F).astype(np.int16)
    g = (shard_of * TILES + tile_of) * P + part_of
    counts = np.bincount(g, minlength=NS * TILES * P)
    C = C_DEFAULT
    if counts.max() > C:                       # safety fallback, recompiles
        C = int(-(-int(counts.max() + 64) // 32) * 32)
    csum = np.zeros(len(counts), np.int64)
    np.cumsum(counts[:-1], out=csum[1:])
    rank = np.arange(len(uniq)) - csum[g]
    idx_all = np.full((NS * TILES * P, C), -1, np.int16)
    idx_all[g, rank] = col_of

    Apad = np.zeros((NS, SPAD), np.float32)
    Bpad = np.ones((NS, SPAD), np.float32)
    for s in range(NS):
        lo = s * S_OWN
        hi = min((s + 1) * S_OWN, n_table)
        Apad[s, : hi - lo] = A[lo:hi]
        Bpad[s, : hi - lo] = B[lo:hi]
    tb = np.ascontiguousarray(np.broadcast_to(t.reshape(-1)[:1, None],
                                              (P, 1))).astype(np.float32)

    in_maps = []
    for s in range(NS):
        in_maps.append({
            "Ain": Apad[s].reshape(TILES * P, F),
            "Bin": Bpad[s].reshape(TILES * P, F),
            "idx": idx_all[s * TILES * P:(s + 1) * TILES * P],
            "tb": tb,
        })

    # ---- run the SPMD bass kernel on cores 0-7 ----
    nc = _get_nc(C)
    global _LAST_IN_MAPS
    _LAST_IN_MAPS = in_maps
    res = bass_utils.run_bass_kernel_spmd(nc, in_maps, core_ids=list(range(NS)))

    # ---- reassemble full outputs; route log-weights back ----
    A_new = np.empty(n_table, np.float32)
    B_new = np.empty(n_table, np.float32)
    llog = np.empty((NS, SPAD), np.float32)
    for s in range(NS):
        lo = s * S_OWN
        hi = min((s + 1) * S_OWN, n_table)
        A_new[lo:hi] = res.results[s]["Aout"].ravel()[: hi - lo]
        B_new[lo:hi] = res.results[s]["Bout"].ravel()[: hi - lo]
        llog[s] = res.results[s]["Llog"].ravel().astype(np.float32)
    vals = llog[shard_of, lu]
    log_w = vals[inverse].astype(np.float32)
    t_new = (t + 1.0).astype(np.float32).reshape(t.shape)
    return log_w, A_new, B_new, t_new


# revision 7
# speedup vs baseline: 1.0058x; 1.0058x over previous
"""EstPop_Debias on 8 Trainium2 NeuronCores.

Strategy (expert/embedding-style parallelism, all-to-all emulated by host
routing): the A/B tables are sharded by item-id range over the 8 cores.
The host routes each sampled item id to its owning shard and converts it
to (tile, partition, column) coordinates; each core streams its table
shard through SBUF in [128 x 1960] tiles and, per tile:

  - builds the updated-slot mask with a GPSIMD local_scatter of the
    routed per-partition column indices,
  - computes delta = (1-a)*B + a*(t1 - A) densely in one fused custom
    DVE op,
  - computes -ln(delta) on the Scalar (ACT) engine,
  - merges delta / t1 into the B / A tiles with predicated copies,
  - streams the merged tiles and the log-weight tile back to HBM.

The host then reassembles the full tables and routes the per-slot log
weights back to the 4M sampled positions (dedup + inverse mapping).
"""

import numpy as np

import concourse.bacc as bacc
import concourse.mybir as mybir
import concourse.tile as tile
from concourse import bass_utils, library_config

ALPHA = 0.0001
ITEM_NUM = 10000001     # table size, matches reference
N_ITEMS = 4194304       # sampled ids per step, matches reference
NS = 8                  # neuron cores
P = 128                 # SBUF partitions
F = 1960                # table columns per partition per tile
TILES = 5               # tiles per shard: 5*128*1960 = 1254400 >= ceil(table/8)
C_DEFAULT = 800         # per-partition routed-index capacity (max observed 761)

AF = mybir.ActivationFunctionType
ALU = mybir.AluOpType

_NC_CACHE = {}
_DELTA_OP = None


def _register_delta_op():
    """Fused EMA delta as a custom DVE op:
    out = s1*in1 + (s0 - imm2*in0), bound as in0=A, in1=B,
    s0 = alpha*(t+1) per-partition scalar, s1 = 1-alpha, imm2 = alpha."""
    global _DELTA_OP
    if _DELTA_OP is not None:
        return _DELTA_OP
    from concourse.dve_ops import (
        OPS, CUSTOM_DVE_SPECS, DveOp, DveOpSpec,
        _CUSTOM_DVE_ROW_BASE, _SUB_OPCODE_FOR_NAME,
    )
    from concourse.dve_spec import Spec, Src0, Src1, C0, C1, C2, lower

    name = "EMA_DELTA_ANT"
    spec = Spec(
        body=Src1 * C1 + (C0 - Src0 * C2),
        reference=lambda in0, in1, s0, s1, imm2: in1 * s1 + (s0 - in0 * imm2),
    )
    if name in _SUB_OPCODE_FOR_NAME:
        _DELTA_OP = next(op for op in OPS if op.name == name)
        return _DELTA_OP
    row = _CUSTOM_DVE_ROW_BASE + len(OPS)
    assert row < 0x20, "custom DVE row table full"
    shas = {}
    for ver in ("v3", "v4"):
        u = lower(spec, ver=ver)
        shas[ver] = DveOpSpec(name=name, opcode=row, uops=u, rd1_en=True).sha(ver)
    op = DveOp(name, spec, subdim=False, uops_sha=shas)
    OPS.append(op)
    CUSTOM_DVE_SPECS[name] = spec
    _SUB_OPCODE_FOR_NAME[name] = row
    _DELTA_OP = op
    return op


def _build_nc(C):
    """One SPMD program; every core runs the same tile loop on its shard."""
    delta_op = _register_delta_op()
    f32, i16, u16, bf16 = (mybir.dt.float32, mybir.dt.int16,
                           mybir.dt.uint16, mybir.dt.bfloat16)
    nc = bacc.Bacc("TRN2", target_bir_lowering=False, debug=False,
                   num_devices=NS)
    Ain_d = nc.dram_tensor("Ain", [TILES * P, F], f32, kind="ExternalInput").ap()
    Bin_d = nc.dram_tensor("Bin", [TILES * P, F], f32, kind="ExternalInput").ap()
    # routed per-partition column ids, pre-transposed on host to [P, TILES*C]
    idx_d = nc.dram_tensor("idx", [P, TILES * C], i16, kind="ExternalInput").ap()
    tb_d = nc.dram_tensor("tb", [P, 1], f32, kind="ExternalInput").ap()
    Aout_d = nc.dram_tensor("Aout", [TILES * P, F], f32, kind="ExternalOutput").ap()
    Bout_d = nc.dram_tensor("Bout", [TILES * P, F], f32, kind="ExternalOutput").ap()
    Llog_d = nc.dram_tensor("Llog", [TILES * P, F], bf16, kind="ExternalOutput").ap()

    with tile.TileContext(nc) as tc:
        with tc.tile_pool(name="const", bufs=1) as cpool, \
             tc.tile_pool(name="work", bufs=4) as pool:
            tb = cpool.tile([P, 1], f32)
            c_ap = cpool.tile([P, 1], f32)
            t1_ap = cpool.tile([P, 1], f32)
            ones_t = cpool.tile([P, C], u16)
            T1t = cpool.tile([P, F], f32)
            idxT = cpool.tile([P, TILES * C], i16)
            nc.sync.dma_start(out=tb[:], in_=tb_d[:])
            nc.sync.dma_start(out=idxT[:], in_=idx_d[:])
            nc.vector.tensor_scalar(out=t1_ap[:], in0=tb[:], scalar1=1.0,
                                    scalar2=None, op0=ALU.add)
            nc.vector.tensor_scalar(out=c_ap[:], in0=t1_ap[:], scalar1=ALPHA,
                                    scalar2=None, op0=ALU.mult)
            nc.vector.memset(ones_t[:], 1)
            nc.vector.memset(T1t[:], 0.0)
            nc.scalar.activation(out=T1t[:], in_=T1t[:], func=AF.Identity,
                                 bias=t1_ap[:], scale=1.0)
            nc.gpsimd.load_library(library_config.local_scatter)

            for tau in range(TILES):
                sl = slice(tau * P, (tau + 1) * P)
                At = pool.tile([P, F], f32)
                Bt = pool.tile([P, F], f32)
                Dt = pool.tile([P, F], f32)
                Lt = pool.tile([P, F], f32)
                Lb = pool.tile([P, F], bf16)
                Mt = pool.tile([P, F], u16)
                nc.sync.dma_start(out=At[:], in_=Ain_d[sl, :])
                nc.sync.dma_start(out=Bt[:], in_=Bin_d[sl, :])
                nc.gpsimd.local_scatter(out_ap=Mt[:], data_ap=ones_t[:],
                                        idxs_ap=idxT[:, tau * C:(tau + 1) * C],
                                        channels=P, num_elems=F, num_idxs=C)
                nc.vector._custom_dve(delta_op, out=Dt[:], in0=At[:], in1=Bt[:],
                                      s0=c_ap[:], s1=1.0 - ALPHA, imm2=ALPHA)
                nc.scalar.activation(out=Lt[:], in_=Dt[:], func=AF.Ln)
                nc.scalar.activation(out=Lb[:], in_=Lt[:], func=AF.Copy,
                                     scale=-1.0)
                nc.vector.copy_predicated(out=Bt[:], mask=Mt[:], data=Dt[:])
                nc.vector.copy_predicated(out=At[:], mask=Mt[:], data=T1t[:])
                nc.scalar.dma_start(out=Aout_d[sl, :], in_=At[:])
                nc.scalar.dma_start(out=Bout_d[sl, :], in_=Bt[:])
                nc.scalar.dma_start(out=Llog_d[sl, :], in_=Lb[:])
    nc.compile()
    return nc


def _get_nc(C):
    nc = _NC_CACHE.get(C)
    if nc is None:
        nc = _build_nc(C)
        _NC_CACHE[C] = nc
    return nc


def kernel(items, A, B, t):
    items = np.asarray(items)
    A = np.asarray(A, dtype=np.float32)
    B = np.asarray(B, dtype=np.float32)
    t = np.asarray(t, dtype=np.float32)
    n_table = A.shape[0]
    assert n_table == ITEM_NUM and B.shape[0] == n_table

    SPAD = TILES * P * F                      # padded slots per shard
    S_OWN = (n_table + NS - 1) // NS          # slots owned per shard
    assert SPAD >= S_OWN

    # ---- host routing: shard by id range, dedup, to local coordinates ----
    uniq, inverse = np.unique(items.astype(np.int64), return_inverse=True)
    assert uniq.min() >= 0 and uniq.max() < n_table
    shard_of = uniq // S_OWN
    lu = uniq - shard_of * S_OWN
    tile_of = lu // (P * F)
    part_of = (lu % (P * F)) // F
    col_of = (lu % F).astype(np.int16)
    g = (shard_of * TILES + tile_of) * P + part_of
    counts = np.bincount(g, minlength=NS * TILES * P)
    C = C_DEFAULT
    if counts.max() > C:                       # safety fallback, recompiles
        C = int(-(-int(counts.max() + 64) // 32) * 32)
    csum = np.zeros(len(counts), np.int64)
    np.cumsum(counts[:-1], out=csum[1:])
    rank = np.arange(len(uniq)) - csum[g]
    idx_all = np.full((NS, TILES, P, C), -1, np.int16)
    idx_all.reshape(-1, C)[g, rank] = col_of
    idx_t = np.ascontiguousarray(
        idx_all.transpose(0, 2, 1, 3).reshape(NS, P, TILES * C))

    Apad = np.zeros((NS, SPAD), np.float32)
    Bpad = np.ones((NS, SPAD), np.float32)
    for s in range(NS):
        lo = s * S_OWN
        hi = min((s + 1) * S_OWN, n_table)
        Apad[s, : hi - lo] = A[lo:hi]
        Bpad[s, : hi - lo] = B[lo:hi]
    tb = np.ascontiguousarray(np.broadcast_to(t.reshape(-1)[:1, None],
                                              (P, 1))).astype(np.float32)

    in_maps = []
    for s in range(NS):
        in_maps.append({
            "Ain": Apad[s].reshape(TILES * P, F),
            "Bin": Bpad[s].reshape(TILES * P, F),
            "idx": idx_t[s],
            "tb": tb,
        })

    # ---- run the SPMD bass kernel on cores 0-7 ----
    nc = _get_nc(C)
    global _LAST_IN_MAPS
    _LAST_IN_MAPS = in_maps
    res = bass_utils.run_bass_kernel_spmd(nc, in_maps, core_ids=list(range(NS)))

    # ---- reassemble full outputs; route log-weights back ----
    A_new = np.empty(n_table, np.float32)
    B_new = np.empty(n_table, np.float32)
    llog = np.empty((NS, SPAD), np.float32)
    for s in range(NS):
        lo = s * S_OWN
        hi = min((s + 1) * S_OWN, n_table)
        A_new[lo:hi] = res.results[s]["Aout"].ravel()[: hi - lo]
        B_new[lo:hi] = res.results[s]["Bout"].ravel()[: hi - lo]
        llog[s] = res.results[s]["Llog"].ravel().astype(np.float32)
    vals = llog[shard_of, lu]
    log_w = vals[inverse].astype(np.float32)
    t_new = (t + 1.0).astype(np.float32).reshape(t.shape)
    return log_w, A_new, B_new, t_new


# revision 8
# speedup vs baseline: 1.0104x; 1.0046x over previous
"""EstPop_Debias on 8 Trainium2 NeuronCores.

Strategy (expert/embedding-style parallelism, all-to-all emulated by host
routing): the A/B tables are sharded by item-id range over the 8 cores.
The host routes each sampled item id to its owning shard and converts it
to (tile, partition, column) coordinates; each core streams its table
shard through SBUF in [128 x 1960] tiles and, per tile:

  - builds the updated-slot mask with a GPSIMD local_scatter of the
    routed per-partition column indices,
  - computes delta = (1-a)*B + a*(t1 - A) densely in one fused custom
    DVE op,
  - computes -ln(delta) on the Scalar (ACT) engine,
  - merges delta / t1 into the B / A tiles with predicated copies,
  - streams the merged tiles and the log-weight tile back to HBM.

The host then reassembles the full tables and routes the per-slot log
weights back to the 4M sampled positions (dedup + inverse mapping).
"""

import numpy as np

import concourse.bacc as bacc
import concourse.mybir as mybir
import concourse.tile as tile
from concourse import bass_utils, library_config

ALPHA = 0.0001
ITEM_NUM = 10000001     # table size, matches reference
N_ITEMS = 4194304       # sampled ids per step, matches reference
NS = 8                  # neuron cores
P = 128                 # SBUF partitions
F = 1960                # table columns per partition per tile
TILES = 5               # tiles per shard: 5*128*1960 = 1254400 >= ceil(table/8)
C_DEFAULT = 800         # per-partition routed-index capacity (max observed 761)

AF = mybir.ActivationFunctionType
ALU = mybir.AluOpType

_NC_CACHE = {}
_DELTA_OP = None


def _register_delta_op():
    """Fused EMA delta as a custom DVE op:
    out = s1*in1 + (s0 - imm2*in0), bound as in0=A, in1=B,
    s0 = alpha*(t+1) per-partition scalar, s1 = 1-alpha, imm2 = alpha."""
    global _DELTA_OP
    if _DELTA_OP is not None:
        return _DELTA_OP
    from concourse.dve_ops import (
        OPS, CUSTOM_DVE_SPECS, DveOp, DveOpSpec,
        _CUSTOM_DVE_ROW_BASE, _SUB_OPCODE_FOR_NAME,
    )
    from concourse.dve_spec import Spec, Src0, Src1, C0, C1, C2, lower

    name = "EMA_DELTA_ANT"
    spec = Spec(
        body=Src1 * C1 + (C0 - Src0 * C2),
        reference=lambda in0, in1, s0, s1, imm2: in1 * s1 + (s0 - in0 * imm2),
    )
    if name in _SUB_OPCODE_FOR_NAME:
        _DELTA_OP = next(op for op in OPS if op.name == name)
        return _DELTA_OP
    row = _CUSTOM_DVE_ROW_BASE + len(OPS)
    assert row < 0x20, "custom DVE row table full"
    shas = {}
    for ver in ("v3", "v4"):
        u = lower(spec, ver=ver)
        shas[ver] = DveOpSpec(name=name, opcode=row, uops=u, rd1_en=True).sha(ver)
    op = DveOp(name, spec, subdim=False, uops_sha=shas)
    OPS.append(op)
    CUSTOM_DVE_SPECS[name] = spec
    _SUB_OPCODE_FOR_NAME[name] = row
    _DELTA_OP = op
    return op


def _build_nc(C):
    """One SPMD program; every core runs the same tile loop on its shard."""
    delta_op = _register_delta_op()
    f32, i16, u16, bf16 = (mybir.dt.float32, mybir.dt.int16,
                           mybir.dt.uint16, mybir.dt.bfloat16)
    nc = bacc.Bacc("TRN2", target_bir_lowering=False, debug=False,
                   num_devices=NS)
    Ain_d = nc.dram_tensor("Ain", [TILES * P, F], f32, kind="ExternalInput").ap()
    Bin_d = nc.dram_tensor("Bin", [TILES * P, F], f32, kind="ExternalInput").ap()
    # routed per-partition column ids, pre-transposed on host to [P, TILES*C]
    idx_d = nc.dram_tensor("idx", [P, TILES * C], i16, kind="ExternalInput").ap()
    tb_d = nc.dram_tensor("tb", [P, 1], f32, kind="ExternalInput").ap()
    Aout_d = nc.dram_tensor("Aout", [TILES * P, F], f32, kind="ExternalOutput").ap()
    Bout_d = nc.dram_tensor("Bout", [TILES * P, F], f32, kind="ExternalOutput").ap()
    Llog_d = nc.dram_tensor("Llog", [TILES * P, F], bf16, kind="ExternalOutput").ap()

    with tile.TileContext(nc) as tc:
        with tc.tile_pool(name="const", bufs=1) as cpool, \
             tc.tile_pool(name="work", bufs=4) as pool:
            tb = cpool.tile([P, 1], f32)
            c_ap = cpool.tile([P, 1], f32)
            t1_ap = cpool.tile([P, 1], f32)
            ones_t = cpool.tile([P, C], u16)
            T1t = cpool.tile([P, F], f32)
            idxT = cpool.tile([P, TILES * C], i16)
            nc.sync.dma_start(out=tb[:], in_=tb_d[:])
            nc.sync.dma_start(out=idxT[:], in_=idx_d[:])
            nc.vector.tensor_scalar(out=t1_ap[:], in0=tb[:], scalar1=1.0,
                                    scalar2=None, op0=ALU.add)
            nc.vector.tensor_scalar(out=c_ap[:], in0=t1_ap[:], scalar1=ALPHA,
                                    scalar2=None, op0=ALU.mult)
            nc.vector.memset(ones_t[:], 1)
            nc.vector.memset(T1t[:], 0.0)
            nc.scalar.activation(out=T1t[:], in_=T1t[:], func=AF.Identity,
                                 bias=t1_ap[:], scale=1.0)
            nc.gpsimd.load_library(library_config.local_scatter)

            for tau in range(TILES):
                sl = slice(tau * P, (tau + 1) * P)
                At = pool.tile([P, F], f32)
                Bt = pool.tile([P, F], f32)
                Dt = pool.tile([P, F], f32)
                Lt = pool.tile([P, F], f32)
                Lb = pool.tile([P, F], bf16)
                Mt = pool.tile([P, F], u16)
                nc.sync.dma_start(out=At[:], in_=Ain_d[sl, :])
                nc.sync.dma_start(out=Bt[:], in_=Bin_d[sl, :])
                nc.gpsimd.local_scatter(out_ap=Mt[:], data_ap=ones_t[:],
                                        idxs_ap=idxT[:, tau * C:(tau + 1) * C],
                                        channels=P, num_elems=F, num_idxs=C)
                nc.vector._custom_dve(delta_op, out=Dt[:], in0=At[:], in1=Bt[:],
                                      s0=c_ap[:], s1=1.0 - ALPHA, imm2=ALPHA)
                # A-merge first: unblocks the Aout DMA one DVE pass earlier
                nc.vector.copy_predicated(out=At[:], mask=Mt[:], data=T1t[:])
                nc.scalar.dma_start(out=Aout_d[sl, :], in_=At[:])
                nc.scalar.activation(out=Lt[:], in_=Dt[:], func=AF.Ln)
                nc.scalar.activation(out=Lb[:], in_=Lt[:], func=AF.Copy,
                                     scale=-1.0)
                nc.scalar.dma_start(out=Llog_d[sl, :], in_=Lb[:])
                nc.vector.copy_predicated(out=Bt[:], mask=Mt[:], data=Dt[:])
                nc.scalar.dma_start(out=Bout_d[sl, :], in_=Bt[:])
    nc.compile()
    return nc


def _get_nc(C):
    nc = _NC_CACHE.get(C)
    if nc is None:
        nc = _build_nc(C)
        _NC_CACHE[C] = nc
    return nc


def kernel(items, A, B, t):
    items = np.asarray(items)
    A = np.asarray(A, dtype=np.float32)
    B = np.asarray(B, dtype=np.float32)
    t = np.asarray(t, dtype=np.float32)
    n_table = A.shape[0]
    assert n_table == ITEM_NUM and B.shape[0] == n_table

    SPAD = TILES * P * F                      # padded slots per shard
    S_OWN = (n_table + NS - 1) // NS          # slots owned per shard
    assert SPAD >= S_OWN

    # ---- host routing: shard by id range, dedup, to local coordinates ----
    uniq, inverse = np.unique(items.astype(np.int64), return_inverse=True)
    assert uniq.min() >= 0 and uniq.max() < n_table
    shard_of = uniq // S_OWN
    lu = uniq - shard_of * S_OWN
    tile_of = lu // (P * F)
    part_of = (lu % (P * F)) // F
    col_of = (lu % F).astype(np.int16)
    g = (shard_of * TILES + tile_of) * P + part_of
    counts = np.bincount(g, minlength=NS * TILES * P)
    C = C_DEFAULT
    if counts.max() > C:                       # safety fallback, recompiles
        C = int(-(-int(counts.max() + 64) // 32) * 32)
    csum = np.zeros(len(counts), np.int64)
    np.cumsum(counts[:-1], out=csum[1:])
    rank = np.arange(len(uniq)) - csum[g]
    idx_all = np.full((NS, TILES, P, C), -1, np.int16)
    idx_all.reshape(-1, C)[g, rank] = col_of
    idx_t = np.ascontiguousarray(
        idx_all.transpose(0, 2, 1, 3).reshape(NS, P, TILES * C))

    Apad = np.zeros((NS, SPAD), np.float32)
    Bpad = np.ones((NS, SPAD), np.float32)
    for s in range(NS):
        lo = s * S_OWN
        hi = min((s + 1) * S_OWN, n_table)
        Apad[s, : hi - lo] = A[lo:hi]
        Bpad[s, : hi - lo] = B[lo:hi]
    tb = np.ascontiguousarray(np.broadcast_to(t.reshape(-1)[:1, None],
                                              (P, 1))).astype(np.float32)

    in_maps = []
    for s in range(NS):
        in_maps.append({
            "Ain": Apad[s].reshape(TILES * P, F),
            "Bin": Bpad[s].reshape(TILES * P, F),
            "idx": idx_t[s],
            "tb": tb,
        })

    # ---- run the SPMD bass kernel on cores 0-7 ----
    nc = _get_nc(C)
    global _LAST_IN_MAPS
    _LAST_IN_MAPS = in_maps
    res = bass_utils.run_bass_kernel_spmd(nc, in_maps, core_ids=list(range(NS)))

    # ---- reassemble full outputs; route log-weights back ----
    A_new = np.empty(n_table, np.float32)
    B_new = np.empty(n_table, np.float32)
    llog = np.empty((NS, SPAD), np.float32)
    for s in range(NS):
        lo = s * S_OWN
        hi = min((s + 1) * S_OWN, n_table)
        A_new[lo:hi] = res.results[s]["Aout"].ravel()[: hi - lo]
        B_new[lo:hi] = res.results[s]["Bout"].ravel()[: hi - lo]
        llog[s] = res.results[s]["Llog"].ravel().astype(np.float32)
    vals = llog[shard_of, lu]
    log_w = vals[inverse].astype(np.float32)
    t_new = (t + 1.0).astype(np.float32).reshape(t.shape)
    return log_w, A_new, B_new, t_new
